# revision 1
# baseline (speedup 1.0000x reference)
"""Trainium2 Bass kernel for nn_MultiHeadAttention_52261162058330.

Reference computes, per (batch, head):
    scores = X @ X.T          # [T, T]
    out    = scores @ X       # [T, D]
with X = x[b, h] of shape [T=2048, D=64], no softmax / no scaling.

Optimizations:
 1. Associativity: out = (X X^T) X = X (X^T X) = X @ G with G = X^T X a
    [64, 64] Gram matrix -> ~32x fewer FLOPs, exact up to summation order.
 2. Split-precision matmuls: X = H + L with H = bf16(X), L = bf16(X - H)
    (covers ~17 mantissa bits).  All matmuls run in bf16 (1 cyc/row on the
    PE + fast weight load vs 4 cyc/row for fp32) accumulating in fp32 PSUM:
      G   = H^T H + H^T L + (H^T L)^T     (drops L^T L ~ 2^-34)
      out = (H + L) @ (Gh + Gl)           (G = Gh + Gl split likewise)
    End-to-end rel error ~ 5e-6 vs the fp32 reference.
 3. Out-stage packing: per row-tile u one [K=128, M=128, N=128] matmul with
    lhsT = [H_u^T ; L_u^T] stacked on K and rhs = [[Gh, Gl], [Gh, Gl]];
    the two N-halves are summed in the epilogue (copy + add).

Sharding: B*H = 32 (batch, head) pairs -> 4 heads per core on 8 cores,
fully independent (no collectives).

Layouts per head (T split as (p u): partition p holds rows 16p..16p+15,
contiguous per partition for DMA):
  hl   [128, 16, 2, 64] bf16 : per tile u: [H_u | L_u]
  xthl [128, 16, 128] bf16   : tile u columns = transpose of [H_u | L_u],
       i.e. rows 0:64 = H_u^T, rows 64:128 = L_u^T
  g2   [128, 2, 64] bf16     : [[Gh, Gl], [Gh, Gl]] (rows duplicated)
"""

import numpy as np

N_CORES = 8
B, H, T, D = 2, 16, 2048, 64
HPC = (B * H) // N_CORES  # heads per core
U = T // 128              # 16 row-tiles per head

_NC = None


def _patch_walrus_flags():
    """Flip --enable-ldw-opt so walrus drops redundant LDWEIGHTS (the
    out-stage issues two matmuls per stationary)."""
    from concourse import bass_utils

    if getattr(bass_utils, "_ldw_patched", False):
        return
    orig = bass_utils.run_command

    def run_command(cmd, *a, **kw):
        if cmd and "walrus_driver" in str(cmd[0]):
            cmd = ["--enable-ldw-opt=true" if c == "--enable-ldw-opt=false" else c
                   for c in cmd]
        return orig(cmd, *a, **kw)

    bass_utils.run_command = run_command
    bass_utils._ldw_patched = True


def _patch_tile_tail():
    """Slim TileContext's exit sequence: drop the second all-engine barrier
    (only needed to fence re-entry, which a kernel tail doesn't have)."""
    from concourse import tile as tile_mod

    if getattr(tile_mod.TileContext, "_tail_patched", False):
        return
    from concourse.tile import ScopedClock

    def _drain_and_barrier(self, tick_clock, wait_clock):
        drain_inst = self.nc.sync.drain()
        wait_clock.add_sem_waits(
            drain_inst.ins, ScopedClock({None: tick_clock.global_clock})
        )
        self.nc.all_engine_barrier()
        popped = self.nc._tile_sem_poison_stack.pop()
        assert popped is self._sem_poison
        self.nc.clear_and_free_semaphores(list(self.sems.allocated().values()))

    tile_mod.TileContext._drain_and_barrier = _drain_and_barrier
    tile_mod.TileContext._tail_patched = True


def _build():
    import concourse.bacc as bacc
    import concourse.mybir as mybir
    from concourse import tile, masks

    _patch_tile_tail()

    nc = bacc.Bacc(
        trn_type="TRN2", target_bir_lowering=False, debug=False,
        num_devices=N_CORES,
    )
    f32 = mybir.dt.float32
    bf16 = mybir.dt.bfloat16
    x_in = nc.dram_tensor("x_shard", [HPC, T, D], f32, kind="ExternalInput").ap()
    y_out = nc.dram_tensor("out_shard", [HPC, T, D], f32, kind="ExternalOutput").ap()
    xv = x_in.rearrange("h (p u) d -> p h u d", p=128)
    yv = y_out.rearrange("h (p u) d -> p h u d", p=128)

    with tile.TileContext(nc) as tc:
        with (
            tc.tile_pool(name="const", bufs=1) as cpool,
            tc.tile_pool(name="iox", bufs=4) as iox,
            tc.tile_pool(name="io", bufs=2) as io,
            tc.tile_pool(name="ios", bufs=4) as ios,
            tc.tile_pool(name="psT", bufs=2, space="PSUM") as psT,
            tc.tile_pool(name="psG", bufs=1, space="PSUM") as psG,
            tc.tile_pool(name="psF", bufs=2, space="PSUM") as psF,
            tc.tile_pool(name="psO", bufs=3, space="PSUM") as psO,
        ):
            identb = cpool.tile([128, 128], bf16)
            masks.make_identity(nc, identb[:])
            identf = cpool.tile([64, 64], f32)
            masks.make_identity(nc, identf[:])

            for h in range(HPC):
                xsb = iox.tile([128, U, D], f32, tag="xsb")
                hl = io.tile([128, U, 2, D], bf16, tag="hl")
                chunks = (0, 8, U) if h == 0 else (0, U)
                for c in range(len(chunks) - 1):
                    sl = slice(chunks[c], chunks[c + 1])
                    nc.sync.dma_start(out=xsb[:, sl], in_=xv[:, h, sl])
                    nc.vector.tensor_copy(hl[:, sl, 0, :], xsb[:, sl])
                    nc.vector.tensor_sub(hl[:, sl, 1, :], xsb[:, sl], hl[:, sl, 0, :])

                # xthl[:, u, :] = [H_u | L_u]^T  (rows 0:64 H^T, 64:128 L^T)
                xthl = io.tile([128, U, 128], bf16, tag="xthl")
                for q in range(U // 4):
                    pst = psT.tile([128, 4, 128], bf16, tag="pst")
                    for i in range(4):
                        u = 4 * q + i
                        nc.tensor.transpose(pst[:, i, :], hl[:, u].rearrange("p a b -> p (a b)"), identb[:])
                    if q % 4 < 3:
                        nc.vector.tensor_copy(xthl[:, 4 * q:4 * q + 4, :], pst[:])
                    else:
                        nc.scalar.copy(xthl[:, 4 * q:4 * q + 4, :], pst[:])

                # G partials: psg[:, 0] = sum H_u^T H_u, psg[:, 1] = sum H_u^T L_u
                psg = psG.tile([64, 2, D], f32, tag="psg")
                for u in range(U):
                    nc.tensor.matmul(
                        psg[:].rearrange("p a b -> p (a b)"),
                        hl[:, u, 0, :],
                        hl[:, u].rearrange("p a b -> p (a b)"),
                        start=(u == 0), stop=(u == U - 1),
                    )
                shl = ios.tile([64, 2, D], f32, tag="shl")
                nc.vector.tensor_copy(shl[:], psg[:])
                # G = HH + HL + HL^T  (HL^T via PE)
                pft = psF.tile([64, D], f32, tag="pf")
                nc.tensor.transpose(pft[:], shl[:, 1, :], identf[:])
                gf = ios.tile([64, D], f32, tag="gf")
                nc.vector.tensor_add(gf[:], shl[:, 0, :], shl[:, 1, :])
                nc.vector.tensor_add(gf[:], gf[:], pft[:])

                # split G; gcat = [Gh | Gl], gcat2 = [Gl | Gh]
                gcat = ios.tile([64, 2, D], bf16, tag="gcat")
                nc.scalar.copy(gcat[:, 0, :], gf[:])
                nc.vector.tensor_sub(gcat[:, 1, :], gf[:], gcat[:, 0, :])
                gcat2 = ios.tile([64, 2, D], bf16, tag="gcat2")
                nc.scalar.copy(gcat2[:, 1, :], gf[:])
                nc.vector.tensor_sub(gcat2[:, 0, :], gf[:], gcat2[:, 1, :])
                # transpose(gcat)  -> rows 64:128 = Gl  (partitions 64:128)
                # transpose(gcat2) -> rows 64:128 = Gh  (partitions 64:128)
                pgt = psF.tile([128, 2, D], bf16, tag="pf")
                nc.tensor.transpose(
                    pgt[:, 0, :], gcat[:].rearrange("p a b -> p (a b)"), identb[0:64, 0:64]
                )
                nc.tensor.transpose(
                    pgt[:, 1, :], gcat2[:].rearrange("p a b -> p (a b)"), identb[0:64, 0:64]
                )
                # g2[:, 0, :] = [Gh; Gh], g2[:, 1, :] = [Gl; Gl] (K-stacked)
                g2 = ios.tile([128, 2, D], bf16, tag="g2")
                nc.vector.tensor_copy(g2[0:64, :, :], gcat[:])
                nc.scalar.copy(g2[64:128, 0, :], pgt[64:128, 1, :])
                nc.scalar.copy(g2[64:128, 1, :], pgt[64:128, 0, :])

                # out tiles: per u one [K=128, M=128, N=128] MM with
                # rhs = [[Gh, Gl], [Gh, Gl]]; halves summed per 4-tile bank
                osb = io.tile([128, U, D], f32, tag="osb")
                for q in range(4):
                    pso = psO.tile([128, 4, 2, D], f32, tag="pso")
                    for i in range(4):
                        u = 4 * q + i
                        nc.tensor.matmul(
                            pso[:, i].rearrange("p a b -> p (a b)"),
                            xthl[:, u, :],
                            g2[:].rearrange("p a b -> p (a b)"),
                            start=True, stop=True,
                        )
                    tmp = ios.tile([128, 4, D], f32, tag="otmp")
                    nc.scalar.copy(tmp[:], pso[:, :, 1, :])
                    nc.vector.tensor_add(
                        osb[:, 4 * q:4 * q + 4, :], pso[:, :, 0, :], tmp[:]
                    )
                    if h == HPC - 1:
                        nc.sync.dma_start(
                            out=yv[:, h, 4 * q:4 * q + 4], in_=osb[:, 4 * q:4 * q + 4]
                        )

                if h != HPC - 1:
                    nc.sync.dma_start(out=yv[:, h], in_=osb[:])

    nc.compile()
    return nc


def _get_nc():
    global _NC
    if _NC is None:
        _NC = _build()
    return _NC


def kernel(x: np.ndarray) -> np.ndarray:
    from concourse.bass_utils import run_bass_kernel_spmd

    assert x.shape == (B, H, T, D), x.shape
    x_flat = np.ascontiguousarray(x.reshape(B * H, T, D), dtype=np.float32)
    in_maps = [
        {"x_shard": np.ascontiguousarray(x_flat[c * HPC:(c + 1) * HPC])}
        for c in range(N_CORES)
    ]
    res = run_bass_kernel_spmd(_get_nc(), in_maps, list(range(N_CORES)))
    out = np.concatenate([res.results[c]["out_shard"] for c in range(N_CORES)], axis=0)
    return out.reshape(B, H, T, D)



# revision 3
# speedup vs baseline: 1.2138x; 1.2138x over previous
"""Trainium2 Bass kernel for nn_MultiHeadAttention_52261162058330.

Reference computes, per (batch, head):
    scores = X @ X.T          # [T, T]
    out    = scores @ X       # [T, D]
with X = x[b, h] of shape [T=2048, D=64], no softmax / no scaling.

Design:
 1. Associativity: out = (X X^T) X = X (X^T X) = X @ G with G = X^T X a
    [64, 64] Gram matrix -> ~32x fewer FLOPs.
 2. Pure bf16 (H = bf16(X)) with fp32 PSUM accumulation: end-to-end rel
    l2 error ~2.8e-3 (gate is 2e-2).  Dropping the old split-precision L
    path halves tensor/vector/scalar work.
 3. Pair-fused PE schedule, per pair q of row-tiles (u=2q, v=2q+1):
      pair = [H_u | H_v]                  [128(T), 128]   (stationary)
      MM_t: pst    = pair^T @ I128        -> [H_u^T; H_v^T]   (transpose)
      MM_g: psg   += pair^T @ pair        -> diag blocks accumulate
                                             G_even (p 0:64) / G_odd (p 64:128)
    Same stationary for both matmuls -> walrus --enable-ldw-opt=true drops
    the second LDWEIGHTS.
 4. Partition fold: G = G_even + G_odd duplicated onto both partition
    halves by ONE matmul with constant J2 = [[I,I],[I,I]]:
      psf[m, n] = sum_k J2[k, m] gsb[k, n] = gsb[m%64, n] + gsb[m%64+64, n]
 5. Out stage, one matmul per pair with block-diagonal rhs:
      pso = xt_q^T @ blockdiag(Gh, Gh)  -> [H_u Gh | H_v Gh]   [128, 128]
 6. Engine split: gpsimd casts f32->bf16 (+memsets), scalar evacuates the
    transpose PSUM banks + issues output DMAs (2nd HWDGE ring), vector
    evacuates the out PSUM banks + small G ops, sync issues input DMAs.

Sharding: B*H = 32 (batch, head) pairs -> 4 heads per core on 8 cores,
fully independent (no collectives).

Layout per head (T split as (p u): partition p holds rows 16p..16p+15,
contiguous per partition for DMA).
"""

import numpy as np

N_CORES = 8
B, H, T, D = 2, 16, 2048, 64
HPC = (B * H) // N_CORES  # heads per core
U = T // 128              # 16 row-tiles per head
NP = U // 2               # 8 pairs per head

_NC = None


def _patch_walrus_flags():
    """Flip --enable-ldw-opt so walrus drops redundant LDWEIGHTS (the
    pair stage issues two matmuls per stationary)."""
    from concourse import bass_utils

    if getattr(bass_utils, "_ldw_patched", False):
        return
    orig = bass_utils.run_command

    def run_command(cmd, *a, **kw):
        if cmd and "walrus_driver" in str(cmd[0]):
            cmd = ["--enable-ldw-opt=true" if c == "--enable-ldw-opt=false" else c
                   for c in cmd]
        return orig(cmd, *a, **kw)

    bass_utils.run_command = run_command
    bass_utils._ldw_patched = True


def _patch_tile_tail():
    """Slim TileContext's exit sequence: drop the second all-engine barrier
    (only needed to fence re-entry, which a kernel tail doesn't have)."""
    from concourse import tile as tile_mod

    if getattr(tile_mod.TileContext, "_tail_patched", False):
        return
    from concourse.tile import ScopedClock

    def _drain_and_barrier(self, tick_clock, wait_clock):
        drain_inst = self.nc.sync.drain()
        wait_clock.add_sem_waits(
            drain_inst.ins, ScopedClock({None: tick_clock.global_clock})
        )
        self.nc.all_engine_barrier()
        popped = self.nc._tile_sem_poison_stack.pop()
        assert popped is self._sem_poison
        self.nc.clear_and_free_semaphores(list(self.sems.allocated().values()))

    tile_mod.TileContext._drain_and_barrier = _drain_and_barrier
    tile_mod.TileContext._tail_patched = True


def _build():
    import concourse.bacc as bacc
    import concourse.mybir as mybir
    from concourse import tile, masks

    _patch_tile_tail()

    nc = bacc.Bacc(
        trn_type="TRN2", target_bir_lowering=False, debug=False,
        num_devices=N_CORES,
    )
    f32 = mybir.dt.float32
    bf16 = mybir.dt.bfloat16
    x_in = nc.dram_tensor("x_shard", [HPC, T, D], f32, kind="ExternalInput").ap()
    y_out = nc.dram_tensor("out_shard", [HPC, T, D], f32, kind="ExternalOutput").ap()
    xv = x_in.rearrange("h (p u) d -> p h u d", p=128)
    yv = y_out.rearrange("h (p u) d -> p h u d", p=128)

    with tile.TileContext(nc) as tc:
        with (
            tc.tile_pool(name="const", bufs=1) as cpool,
            tc.tile_pool(name="xin", bufs=2) as xpool,
            tc.tile_pool(name="hbuf", bufs=2) as hpool,
            tc.tile_pool(name="xt", bufs=2) as tpool,
            tc.tile_pool(name="gsm", bufs=2) as gpool,
            tc.tile_pool(name="osb", bufs=2) as opool,
            tc.tile_pool(name="psT", bufs=2, space="PSUM") as psT,
            tc.tile_pool(name="psG", bufs=2, space="PSUM") as psG,
            tc.tile_pool(name="psF", bufs=2, space="PSUM") as psF,
            tc.tile_pool(name="psO", bufs=2, space="PSUM") as psO,
        ):
            identb = cpool.tile([128, 128], bf16)
            masks.make_identity(nc, identb[:])
            # J2[k, m] = 1 iff k % 64 == m % 64  ([[I,I],[I,I]] stacked)
            j2 = cpool.tile([128, 128], bf16)
            nc.gpsimd.memset(j2[:], 0.0)
            for base in (0, -64, 64):
                nc.gpsimd.affine_select(
                    out=j2[:], in_=j2[:],
                    compare_op=mybir.AluOpType.not_equal,
                    fill=1.0, base=base,
                    pattern=[[-1, 128]], channel_multiplier=1,
                )

            for h in range(HPC):
                xsb = xpool.tile([128, U, D], f32, tag="xsb")
                hb = hpool.tile([128, U, D], bf16, tag="hb")
                chunks = (0, 8, U) if h == 0 else (0, U)
                for c in range(len(chunks) - 1):
                    sl = slice(chunks[c], chunks[c + 1])
                    nc.sync.dma_start(out=xsb[:, sl], in_=xv[:, h, sl])
                    nc.gpsimd.tensor_copy(hb[:, sl], xsb[:, sl])

                # pair stage: transpose + Gram, shared stationary per pair
                xt = tpool.tile([128, NP, 128], bf16, tag="xt")
                psg = psG.tile([128, 128], f32, tag="psg")
                for half in range(2):
                    pst = psT.tile([128, 4, 128], f32, tag="pst")
                    for i in range(4):
                        q = 4 * half + i
                        pair = hb[:, 2 * q:2 * q + 2].rearrange("p a b -> p (a b)")
                        nc.tensor.matmul(pst[:, i, :], pair, identb[:],
                                         start=True, stop=True)
                        nc.tensor.matmul(psg[:], pair, pair,
                                         start=(q == 0), stop=(q == NP - 1),
                                         skip_group_check=True)
                    nc.scalar.copy(xt[:, 4 * half:4 * half + 4, :], pst[:])

                # G = G_even + G_odd, duplicated to both partition halves
                gsb = gpool.tile([128, D], bf16, tag="gsb")
                nc.vector.tensor_copy(gsb[0:64, :], psg[0:64, 0:64])
                nc.vector.tensor_copy(gsb[64:128, :], psg[64:128, 64:128])
                psf = psF.tile([128, D], f32, tag="psf")
                nc.tensor.matmul(psf[:], j2[:], gsb[:], start=True, stop=True)
                g2blk = gpool.tile([128, 2, D], bf16, tag="g2blk")
                nc.gpsimd.memset(g2blk[:], 0.0)
                nc.vector.tensor_copy(g2blk[0:64, 0, :], psf[0:64, :])
                nc.vector.tensor_copy(g2blk[64:128, 1, :], psf[64:128, :])
                g2m = g2blk.rearrange("p a b -> p (a b)")

                # out stage: one matmul per pair, rhs = blockdiag(Gh, Gh)
                osb = opool.tile([128, U, D], f32, tag="osb")
                for half in range(2):
                    pso = psO.tile([128, 4, 128], f32, tag="pso")
                    for i in range(4):
                        q = 4 * half + i
                        nc.tensor.matmul(pso[:, i, :], xt[:, q, :], g2m,
                                         start=True, stop=True)
                    nc.vector.tensor_copy(
                        osb[:, 8 * half:8 * half + 8].rearrange("p a b -> p (a b)"),
                        pso[:].rearrange("p a b -> p (a b)"),
                    )

                nc.scalar.dma_start(out=yv[:, h], in_=osb[:])

    nc.compile()
    return nc


def _get_nc():
    global _NC
    if _NC is None:
        _NC = _build()
    return _NC


def kernel(x: np.ndarray) -> np.ndarray:
    from concourse.bass_utils import run_bass_kernel_spmd

    assert x.shape == (B, H, T, D), x.shape
    x_flat = np.ascontiguousarray(x.reshape(B * H, T, D), dtype=np.float32)
    in_maps = [
        {"x_shard": np.ascontiguousarray(x_flat[c * HPC:(c + 1) * HPC])}
        for c in range(N_CORES)
    ]
    res = run_bass_kernel_spmd(_get_nc(), in_maps, list(range(N_CORES)))
    out = np.concatenate([res.results[c]["out_shard"] for c in range(N_CORES)], axis=0)
    return out.reshape(B, H, T, D)


# revision 6
# speedup vs baseline: 1.4399x; 1.1862x over previous
"""Trainium2 Bass kernel for nn_MultiHeadAttention_52261162058330.

Reference computes, per (batch, head):
    scores = X @ X.T          # [T, T]
    out    = scores @ X       # [T, D]
with X = x[b, h] of shape [T=2048, D=64], no softmax / no scaling.

Design:
 1. Associativity: out = (X X^T) X = X (X^T X) = X @ G with G = X^T X a
    [64, 64] Gram matrix -> ~32x fewer FLOPs.
 2. Pure bf16 (H = bf16(X)) with fp32 PSUM accumulation: end-to-end rel
    l2 error ~2.8e-3 (gate is 2e-2).  Dropping the old split-precision L
    path halves tensor/vector/scalar work.
 3. Pair-fused PE schedule, per pair q of row-tiles (u=2q, v=2q+1):
      pair = [H_u | H_v]                  [128(T), 128]   (stationary)
      MM_t: pst    = pair^T @ I128        -> [H_u^T; H_v^T]   (transpose)
      MM_g: psg   += pair^T @ pair        -> diag blocks accumulate
                                             G_even (p 0:64) / G_odd (p 64:128)
    Same stationary for both matmuls -> walrus --enable-ldw-opt=true drops
    the second LDWEIGHTS.
 4. Partition fold: G = G_even + G_odd duplicated onto both partition
    halves by ONE matmul with constant J2 = [[I,I],[I,I]]:
      psf[m, n] = sum_k J2[k, m] gsb[k, n] = gsb[m%64, n] + gsb[m%64+64, n]
 5. Out stage, one matmul per pair with block-diagonal rhs:
      pso = xt_q^T @ blockdiag(Gh, Gh)  -> [H_u Gh | H_v Gh]   [128, 128]
 6. Engine split: gpsimd casts f32->bf16 (+memsets), scalar evacuates the
    transpose PSUM banks + issues output DMAs (2nd HWDGE ring), vector
    evacuates the out PSUM banks + small G ops, sync issues input DMAs.

Sharding: B*H = 32 (batch, head) pairs -> 4 heads per core on 8 cores,
fully independent (no collectives).

Layout per head (T split as (p u): partition p holds rows 16p..16p+15,
contiguous per partition for DMA).
"""

import numpy as np

N_CORES = 8
B, H, T, D = 2, 16, 2048, 64
HPC = (B * H) // N_CORES  # heads per core
U = T // 128              # 16 row-tiles per head
NP = U // 2               # 8 pairs per head

_NC = None


def _patch_walrus_flags():
    """Flip --enable-ldw-opt so walrus drops redundant LDWEIGHTS (the
    pair stage issues two matmuls per stationary)."""
    from concourse import bass_utils

    if getattr(bass_utils, "_ldw_patched", False):
        return
    orig = bass_utils.run_command

    def run_command(cmd, *a, **kw):
        if cmd and "walrus_driver" in str(cmd[0]):
            cmd = ["--enable-ldw-opt=true" if c == "--enable-ldw-opt=false" else c
                   for c in cmd]
        return orig(cmd, *a, **kw)

    bass_utils.run_command = run_command
    bass_utils._ldw_patched = True


def _patch_tile_tail():
    """Slim TileContext's exit sequence: drop the second all-engine barrier
    (only needed to fence re-entry, which a kernel tail doesn't have)."""
    from concourse import tile as tile_mod

    if getattr(tile_mod.TileContext, "_tail_patched", False):
        return
    from concourse.tile import ScopedClock

    def _drain_and_barrier(self, tick_clock, wait_clock):
        drain_inst = self.nc.sync.drain()
        wait_clock.add_sem_waits(
            drain_inst.ins, ScopedClock({None: tick_clock.global_clock})
        )
        self.nc.all_engine_barrier()
        popped = self.nc._tile_sem_poison_stack.pop()
        assert popped is self._sem_poison
        self.nc.clear_and_free_semaphores(list(self.sems.allocated().values()))

    tile_mod.TileContext._drain_and_barrier = _drain_and_barrier
    tile_mod.TileContext._tail_patched = True


def _build():
    import concourse.bacc as bacc
    import concourse.mybir as mybir
    from concourse import tile, masks

    _patch_tile_tail()

    nc = bacc.Bacc(
        trn_type="TRN2", target_bir_lowering=False, debug=False,
        num_devices=N_CORES,
    )
    f32 = mybir.dt.float32
    bf16 = mybir.dt.bfloat16
    x_in = nc.dram_tensor("x_shard", [HPC, T, D], f32, kind="ExternalInput").ap()
    y_out = nc.dram_tensor("out_shard", [HPC, T, D], f32, kind="ExternalOutput").ap()
    xv = x_in.rearrange("h (p u) d -> p h u d", p=128)
    yv = y_out.rearrange("h (p u) d -> p h u d", p=128)

    with tile.TileContext(nc) as tc:
        with (
            tc.tile_pool(name="const", bufs=1) as cpool,
            tc.tile_pool(name="hbuf", bufs=2) as hpool,
            tc.tile_pool(name="xt", bufs=2) as tpool,
            tc.tile_pool(name="gsm", bufs=2) as gpool,
            tc.tile_pool(name="osb", bufs=2) as opool,
            tc.tile_pool(name="psT", bufs=2, space="PSUM") as psT,
            tc.tile_pool(name="psG", bufs=2, space="PSUM") as psG,
            tc.tile_pool(name="psF", bufs=2, space="PSUM") as psF,
            tc.tile_pool(name="psO", bufs=2, space="PSUM") as psO,
        ):
            identb = cpool.tile([128, 128], bf16)
            masks.make_identity(nc, identb[:])
            # J2[k, m] = 1 iff k % 64 == m % 64  ([[I,I],[I,I]] stacked)
            j2 = cpool.tile([128, 128], bf16)
            nc.gpsimd.memset(j2[:], 0.0)
            for base in (0, -64, 64):
                nc.gpsimd.affine_select(
                    out=j2[:], in_=j2[:],
                    compare_op=mybir.AluOpType.not_equal,
                    fill=1.0, base=base,
                    pattern=[[-1, 128]], channel_multiplier=1,
                )

            for h in range(HPC):
                hb = hpool.tile([128, U, D], bf16, tag="hb")
                chunks = (0, 8, U) if h == 0 else (0, U)
                for c in range(len(chunks) - 1):
                    sl = slice(chunks[c], chunks[c + 1])
                    # SWDGE cast-DMA: f32 HBM -> bf16 SBUF inline
                    nc.gpsimd.dma_start(out=hb[:, sl], in_=xv[:, h, sl])

                # pair stage: transpose + Gram, shared stationary per pair
                xt = tpool.tile([128, NP, 128], bf16, tag="xt")
                psg = psG.tile([128, 128], f32, tag="psg")
                for half in range(2):
                    pst = psT.tile([128, 4, 128], f32, tag="pst")
                    for i in range(4):
                        q = 4 * half + i
                        pair = hb[:, 2 * q:2 * q + 2].rearrange("p a b -> p (a b)")
                        nc.tensor.matmul(pst[:, i, :], pair, identb[:],
                                         start=True, stop=True)
                        nc.tensor.matmul(psg[:], pair, pair,
                                         start=(q == 0), stop=(q == NP - 1),
                                         skip_group_check=True)
                    if half == 0:
                        nc.scalar.copy(xt[:, 0:4, :], pst[:])
                    else:
                        nc.vector.tensor_copy(xt[:, 4:8, :], pst[:])

                # G = G_even + G_odd, duplicated to both partition halves
                gsb = gpool.tile([128, D], bf16, tag="gsb")
                nc.vector.tensor_copy(gsb[0:64, :], psg[0:64, 0:64])
                nc.scalar.copy(gsb[64:128, :], psg[64:128, 64:128])
                psf = psF.tile([128, D], f32, tag="psf")
                nc.tensor.matmul(psf[:], j2[:], gsb[:], start=True, stop=True)
                g2blk = gpool.tile([128, 2, D], bf16, tag="g2blk")
                nc.gpsimd.memset(g2blk[:], 0.0)
                nc.vector.tensor_copy(g2blk[0:64, 0, :], psf[0:64, :])
                nc.scalar.copy(g2blk[64:128, 1, :], psf[64:128, :])
                g2m = g2blk.rearrange("p a b -> p (a b)")

                # out stage: one matmul per pair, rhs = blockdiag(Gh, Gh)
                osb = opool.tile([128, U, D], f32, tag="osb")
                for half in range(2):
                    pso = psO.tile([128, 4, 128], f32, tag="pso")
                    for i in range(4):
                        q = 4 * half + i
                        nc.tensor.matmul(pso[:, i, :], xt[:, q, :], g2m,
                                         start=True, stop=True)
                    if half == 0:
                        nc.vector.tensor_copy(
                            osb[:, 0:8].rearrange("p a b -> p (a b)"),
                            pso[:].rearrange("p a b -> p (a b)"))
                    else:
                        nc.scalar.copy(
                            osb[:, 8:16].rearrange("p a b -> p (a b)"),
                            pso[:].rearrange("p a b -> p (a b)"))

                nc.sync.dma_start(out=yv[:, h], in_=osb[:])

    nc.compile()
    return nc


def _get_nc():
    global _NC
    if _NC is None:
        _NC = _build()
    return _NC


def kernel(x: np.ndarray) -> np.ndarray:
    from concourse.bass_utils import run_bass_kernel_spmd

    assert x.shape == (B, H, T, D), x.shape
    x_flat = np.ascontiguousarray(x.reshape(B * H, T, D), dtype=np.float32)
    in_maps = [
        {"x_shard": np.ascontiguousarray(x_flat[c * HPC:(c + 1) * HPC])}
        for c in range(N_CORES)
    ]
    res = run_bass_kernel_spmd(_get_nc(), in_maps, list(range(N_CORES)))
    out = np.concatenate([res.results[c]["out_shard"] for c in range(N_CORES)], axis=0)
    return out.reshape(B, H, T, D)
